# revision 2
# baseline (speedup 1.0000x reference)
"""BiDAF attention Bass kernel for Trainium2 (8 NeuronCores, batch-parallel).

Takes FULL inputs (BS=32, MCL=1024, MQL=64, d=512), shards batch across the
8 cores (4 batches/core), runs one SPMD Bass kernel, gathers the full output
(32, 1024, 2048) float32.

v2: fp16 end-to-end on device (host casts in/out), S computed c-partitioned
directly (no S^T round-trip), fp16 PE transposes with 2x DVE copies, products
split DVE/GpSimd, stores on the ACT HWDGE ring, loads on the SP ring.

Self-contained: only imports concourse (available on sys.path in the
container via sitecustomize).
"""
import sys

if "/opt/trn_rl_repo" not in sys.path:
    sys.path.insert(0, "/opt/trn_rl_repo")

from contextlib import ExitStack

import numpy as np

import concourse.bass as bass
import concourse.bacc as bacc
import concourse.tile as tile
from concourse import mybir

dt = mybir.dt
AF = mybir.ActivationFunctionType
ALU = mybir.AluOpType
AX = mybir.AxisListType

NCORES = 8
BS, MCL, MQL, D = 32, 1024, 64, 512
BPC = BS // NCORES          # batches per core
NT = MCL // 128             # c-tiles per batch
NK = D // 128               # contraction chunks
F32 = dt.float32
F16 = dt.float16
EXP_BIAS = -3.0             # constant shift inside c2q softmax (exact in softmax math)


def build_nc():
    nc = bacc.Bacc("TRN2", target_bir_lowering=False)
    hq_d = nc.dram_tensor("hq", [BPC, MQL, D], F16, kind="ExternalInput")
    hc_d = nc.dram_tensor("hc", [BPC, MCL, D], F16, kind="ExternalInput")
    w_d = nc.dram_tensor("w", [3 * D, 1], F32, kind="ExternalInput")
    id_d = nc.dram_tensor("idm", [128, 128], F16, kind="ExternalInput")
    out_d = nc.dram_tensor("out", [BPC, MCL, 4 * D], F16, kind="ExternalOutput")

    with tile.TileContext(nc) as tc, ExitStack() as ctx:
        const = ctx.enter_context(tc.tile_pool(name="const", bufs=1))
        sb = ctx.enter_context(tc.tile_pool(name="sb", bufs=2))
        sbo = ctx.enter_context(tc.tile_pool(name="sbo", bufs=3))
        ps = ctx.enter_context(tc.tile_pool(name="ps", bufs=1, space="PSUM"))
        psA = ctx.enter_context(tc.tile_pool(name="psA", bufs=2, space="PSUM"))
        psT = ctx.enter_context(tc.tile_pool(name="psT", bufs=2, space="PSUM"))

        # ---- constants ----
        ident = const.tile([128, 128], F16)
        nc.sync.dma_start(ident[:], id_d[:])
        # W as (128, 12): col j holds W[j*128:(j+1)*128]; j=0..3 Wc, 4..7 Wq, 8..11 Wm
        wv = const.tile([128, 12], F32)
        nc.sync.dma_start(wv[:], w_d.rearrange("(j p) o -> p (j o)", p=128))
        wv16 = const.tile([128, 12], F16)
        nc.vector.tensor_copy(wv16[:], wv[:])
        ones_row = const.tile([1, 128], F16)
        nc.vector.memset(ones_row[:], 1.0)
        ones_col = const.tile([128, 1], F16)
        nc.vector.memset(ones_col[:], 1.0)
        bias_e = const.tile([128, 1], F32)
        nc.vector.memset(bias_e[:], EXP_BIAS)
        bias_0 = const.tile([128, 1], F32)
        nc.vector.memset(bias_0[:], 0.0)
        # hq for all batches: (64, BPC, 512)
        hq_all = const.tile([MQL, BPC, D], F16)
        nc.sync.dma_start(hq_all[:], hq_d.rearrange("b q d -> q b d"))

        st = [dict() for _ in range(BPC)]   # per-batch live tiles

        def s1(b):
            """loads, hqT/stw/sq, Hc transposes, S matmuls (c-partitioned)."""
            v = st[b]
            hc_nat = sb.tile([128, NT, D], F16, tag="hc")
            hc_src = hc_d[b].rearrange("(t p) d -> p t d", p=128)
            for hf in range(2):
                nc.sync.dma_start(hc_nat[:, hf * 4:(hf + 1) * 4, :],
                                  hc_src[:, hf * 4:(hf + 1) * 4, :])
            v["hc_nat"] = hc_nat
            hq_r = hq_all[:, b, :]
            v["hq_r"] = hq_r

            # hqT: (128, NK, MQL) via 4 PE transposes
            hqT_ps = psT.tile([128, NK, MQL], F16, tag="t")
            for k in range(NK):
                nc.tensor.transpose(
                    hqT_ps[:, k, :], hq_r[:, k * 128:(k + 1) * 128],
                    ident[0:MQL, 0:MQL])
            hqT_s = sb.tile([128, NK, MQL], F16, tag="hqT")
            nc.vector.tensor_copy(hqT_s[:], hqT_ps[:])

            # stw: cols 0:64 = hqT * Wm_k, col 64 = Wc_k
            stw = sb.tile([128, NK, MQL + 1], F16, tag="stw")
            for k in range(NK):
                nc.vector.tensor_scalar(
                    stw[:, k, 0:MQL], hqT_s[:, k, :],
                    wv[:, 8 + k, None], None, op0=ALU.mult)
            nc.vector.tensor_copy(
                stw[:, :, MQL], wv16[:, 0:NK])

            # sq[q] = sum_d Wq[d] Hq[q, d]  -> (1, MQL) fp32
            sq_ps = psT.tile([1, MQL], F32, tag="t")
            for k in range(NK):
                nc.tensor.matmul(sq_ps[:], wv16[:, 4 + k, None], hqT_s[:, k, :],
                                 start=(k == 0), stop=(k == NK - 1))
            sq_aug = sb.tile([1, MQL + 1], F16, tag="sq_aug")
            nc.vector.memset(sq_aug[:], 0.0)
            nc.vector.tensor_copy(sq_aug[0:1, 0:MQL], sq_ps[:])

            # hcT: (128, NK, MCL) f16 via PE transposes + 2x DVE copies
            hcT = sb.tile([128, NK, MCL], F16, tag="hcT")
            for g in range(8):
                k, hf = divmod(g, 2)
                trp = psT.tile([128, 512], F16, tag="t")
                for j in range(4):
                    t = hf * 4 + j
                    nc.tensor.transpose(
                        trp[:, j * 128:(j + 1) * 128],
                        hc_nat[:, t, k * 128:(k + 1) * 128], ident[:])
                nc.vector.tensor_copy(hcT[:, k, hf * 512:(hf + 1) * 512], trp[:])
            v["hcT"] = hcT

            # S (c-partitioned): S[:, t, 0:65]; col 64 = sc
            S_ps = ps.tile([128, NT, 128], F32, tag="S")
            for t in range(NT):
                for k in range(NK):
                    nc.tensor.matmul(
                        S_ps[:, t, 0:MQL + 1],
                        hcT[:, k, t * 128:(t + 1) * 128], stw[:, k, :],
                        start=(k == 0), stop=False)
                # rank-1: add sq[q] to every c row (col 64 gets 0)
                nc.tensor.matmul(S_ps[:, t, 0:MQL + 1], ones_row[:], sq_aug[:],
                                 start=False, stop=True)
            v["S_ps"] = S_ps

        def s2(b):
            """softmax stats + q2c chain + wT transposes."""
            v = st[b]
            S_ps, hc_nat = v["S_ps"], v["hc_nat"]
            # c2q: E = exp(s - 3), dens, En
            E = sb.tile([128, NT, MQL], F16, tag="E")
            nc.scalar.activation(E[:], S_ps[:, :, 0:MQL], AF.Exp,
                                 bias=bias_e[:], scale=1.0)
            mx = sb.tile([128, NT], F32, tag="mx")
            nc.vector.tensor_reduce(mx[:], S_ps[:, :, 0:MQL],
                                    axis=AX.X, op=ALU.max)
            dens = sb.tile([128, NT], F32, tag="dens")
            nc.vector.tensor_reduce(dens[:], E[:], axis=AX.X, op=ALU.add)
            rec = sb.tile([128, NT], F32, tag="rec")
            nc.vector.reciprocal(rec[:], dens[:])
            En = sb.tile([128, NT, MQL], F16, tag="En")
            nc.vector.tensor_tensor(
                En[:], E[:], rec[:, :, None].broadcast_to((128, NT, MQL)),
                op=ALU.mult)
            # q2c: score = mx + sc; e2 = exp(score); dsum = row sums
            score = sb.tile([128, NT], F32, tag="score")
            nc.vector.tensor_tensor(score[:], mx[:], S_ps[:, :, MQL], op=ALU.add)
            e2 = sb.tile([128, NT], F16, tag="e2")
            dsum = sb.tile([128, 1], F32, tag="dsum")
            nc.scalar.activation(e2[:], score[:], AF.Exp, bias=bias_0[:],
                                 scale=1.0, accum_out=dsum[:])
            dsum16 = sb.tile([128, 1], F16, tag="dsum16")
            nc.vector.tensor_copy(dsum16[:], dsum[:])
            den2_ps = psT.tile([1, 1], F32, tag="t")
            nc.tensor.matmul(den2_ps[:], dsum16[:], ones_col[:],
                             start=True, stop=True)
            rec2 = sb.tile([1, 1], F32, tag="rec2")
            nc.vector.reciprocal(rec2[:], den2_ps[:])
            U_ps = psT.tile([1, D], F32, tag="t")
            for t in range(NT):
                nc.tensor.matmul(U_ps[:], e2[:, t, None], hc_nat[:, t, :],
                                 start=(t == 0), stop=(t == NT - 1))
            qacT = sb.tile([1, D], F16, tag="qacT")
            nc.vector.tensor_scalar(qacT[:], U_ps[:], rec2[:], None, op0=ALU.mult)
            qacB = sb.tile([128, D], F16, tag="qacB")
            nc.gpsimd.partition_broadcast(qacB[:], qacT[:])
            v["qacB"] = qacB

            # wT: (64, NT, 128) f16 = En^T via PE transposes + 2x copies
            wT = sb.tile([MQL, NT, 128], F16, tag="wT")
            for j in range(2):
                wT_ps = psA.tile([MQL, 4, 128], F16, tag="A")
                for i in range(4):
                    t = j * 4 + i
                    nc.tensor.transpose(wT_ps[:, i, :], En[:, t, :], ident[:])
                nc.vector.tensor_copy(wT[:, j * 4:(j + 1) * 4, :], wT_ps[:])
            v["wT"] = wT

        def s3(b):
            """A matmuls + products + tanh + store, per half (4 c-tiles)."""
            v = st[b]
            hq_r, hc_nat, qacB, wT = (v["hq_r"], v["hc_nat"],
                                      v["qacB"], v["wT"])
            out_view = out_d[b].rearrange("(t p) j -> p t j", p=128)
            for h in range(2):
                out_t = sbo.tile([128, 4, 4 * D], F16, tag="out")
                prod = sb.tile([128, 4, 2 * D], F16, tag="prod")
                for i in range(2):
                    A_ps = psA.tile([128, 2, D], F32, tag="A")
                    for tt in range(2):
                        t = h * 4 + i * 2 + tt
                        nc.tensor.matmul(A_ps[:, tt, :], wT[:, t, :], hq_r[:],
                                         start=True, stop=True)
                    nc.scalar.activation(out_t[:, i * 2:(i + 1) * 2, D:2 * D],
                                         A_ps[:], AF.Tanh, bias=bias_0[:],
                                         scale=1.0)
                    nc.vector.tensor_tensor(
                        prod[:, i * 2:(i + 1) * 2, 0:D], A_ps[:],
                        hc_nat[:, h * 4 + i * 2:h * 4 + (i + 1) * 2, :],
                        op=ALU.mult)
                    for tt in range(2):
                        t = h * 4 + i * 2 + tt
                        nc.gpsimd.tensor_tensor(
                            prod[:, i * 2 + tt, D:2 * D], hc_nat[:, t, :],
                            qacB[:], op=ALU.mult)
                nc.scalar.activation(out_t[:, :, 0:D],
                                     hc_nat[:, h * 4:(h + 1) * 4, :],
                                     AF.Tanh, bias=bias_0[:], scale=1.0)
                nc.scalar.activation(out_t[:, :, 2 * D:4 * D], prod[:],
                                     AF.Tanh, bias=bias_0[:], scale=1.0)
                nc.scalar.dma_start(out_view[:, h * 4:(h + 1) * 4, :], out_t[:])

        # software pipeline across batches
        s1(0)
        s1(1)
        s2(0)
        s3(0)
        s1(2)
        s2(1)
        s3(1)
        s1(3)
        s2(2)
        s3(2)
        s2(3)
        s3(3)
    nc.compile()
    return nc


_NC = None


def _get_nc():
    global _NC
    if _NC is None:
        _NC = build_nc()
    return _NC


def run(inputs: dict, trace: bool = False, tmpdir: str | None = None):
    """Shard, run on 8 cores, gather. Returns (out, BassKernelResults)."""
    from concourse.bass_utils import run_bass_kernel_spmd

    if trace:
        # the axon NTFF hook module is absent in this image; inject it
        try:
            from antenv import axon_hooks  # noqa: F401
        except ImportError:
            import types
            import antenv
            from trn_agent_boot.trn_boot import _ntff_profile_via_ctypes
            mod = types.ModuleType("antenv.axon_hooks")
            _hook = _ntff_profile_via_ctypes('/opt/axon/libaxon_pjrt.so')
            mod.get_axon_ntff_profile_hook = lambda: _hook
            mod.set_axon_ntff_profile_hook = lambda h: None
            sys.modules["antenv.axon_hooks"] = mod
            antenv.axon_hooks = mod

    Hq = np.asarray(inputs["Hq"], dtype=np.float16)
    Hc = np.asarray(inputs["Hc"], dtype=np.float16)
    W = np.ascontiguousarray(np.asarray(inputs["W"], dtype=np.float32))
    IDM = np.eye(128, dtype=np.float16)
    nc = _get_nc()
    in_maps = [
        {"hq": np.ascontiguousarray(Hq[i * BPC:(i + 1) * BPC]),
         "hc": np.ascontiguousarray(Hc[i * BPC:(i + 1) * BPC]),
         "w": W, "idm": IDM}
        for i in range(NCORES)
    ]
    br = run_bass_kernel_spmd(nc, in_maps, list(range(NCORES)), trace=trace,
                              tmpdir=tmpdir)
    out = np.concatenate([br.results[i]["out"] for i in range(NCORES)],
                         axis=0).astype(np.float32)
    return out, br


def kernel(**inputs) -> np.ndarray:
    out, _ = run(inputs, trace=False)
    return out


# revision 5
# speedup vs baseline: 1.2298x; 1.2298x over previous
"""BiDAF attention Bass kernel for Trainium2 (8 NeuronCores, batch-parallel).

Takes FULL inputs (BS=32, MCL=1024, MQL=64, d=512), shards batch across the
8 cores (4 batches/core), runs one SPMD Bass kernel, gathers the full output
(32, 1024, 2048) float32.

v3: fp16 on device (host casts in/out). S^T via big-N matmuls; softmax stats
from PE-transposed S tiles; paired En transposes + row-packed A matmuls using
a partition-duplicated Hq; products split DVE (Hc*A from PSUM) / GpSimd
(Hc*U); q2c normalization folded into the tanh scale to shorten the chain;
all loads issued upfront on the SP HWDGE ring, stores on the ACT HWDGE ring.

Self-contained: only imports concourse (available on sys.path in the
container via sitecustomize).
"""
import sys

if "/opt/trn_rl_repo" not in sys.path:
    sys.path.insert(0, "/opt/trn_rl_repo")

from contextlib import ExitStack

import numpy as np

import concourse.bass as bass
import concourse.bacc as bacc
import concourse.tile as tile
from concourse import mybir

dt = mybir.dt
AF = mybir.ActivationFunctionType
ALU = mybir.AluOpType
AX = mybir.AxisListType

NCORES = 8
BS, MCL, MQL, D = 32, 1024, 64, 512
BPC = BS // NCORES          # batches per core
NT = MCL // 128             # c-tiles per batch
NK = D // 128               # contraction chunks
F32 = dt.float32
F16 = dt.float16
EXP_BIAS = -3.0             # constant shift inside c2q softmax (exact in softmax math)


def build_nc():
    nc = bacc.Bacc("TRN2", target_bir_lowering=False)
    hq_d = nc.dram_tensor("hq", [BPC, MQL, D], F16, kind="ExternalInput")
    hc_d = nc.dram_tensor("hc", [BPC, MCL, D], F16, kind="ExternalInput")
    w_d = nc.dram_tensor("w", [3 * D, 1], F32, kind="ExternalInput")
    id_d = nc.dram_tensor("idm", [128, 128], F16, kind="ExternalInput")
    out_d = nc.dram_tensor("out", [BPC, MCL, 4 * D], F16, kind="ExternalOutput")

    with tile.TileContext(nc) as tc, ExitStack() as ctx:
        const = ctx.enter_context(tc.tile_pool(name="const", bufs=1))
        sb = ctx.enter_context(tc.tile_pool(name="sb", bufs=2))
        sbc = ctx.enter_context(tc.tile_pool(name="sbc", bufs=4))
        sbo = ctx.enter_context(tc.tile_pool(name="sbo", bufs=3))
        ps = ctx.enter_context(tc.tile_pool(name="ps", bufs=1, space="PSUM"))
        psA = ctx.enter_context(tc.tile_pool(name="psA", bufs=2, space="PSUM"))
        psT = ctx.enter_context(tc.tile_pool(name="psT", bufs=2, space="PSUM"))

        # ---- constants ----
        ident = const.tile([128, 128], F16)
        nc.sync.dma_start(ident[:], id_d[:])
        # W as (128, 12): col j holds W[j*128:(j+1)*128]; j=0..3 Wc, 4..7 Wq, 8..11 Wm
        wv = const.tile([128, 12], F32)
        nc.sync.dma_start(wv[:], w_d.rearrange("(j p) o -> p (j o)", p=128))
        wv16 = const.tile([128, 12], F16)
        nc.vector.tensor_copy(wv16[:], wv[:])
        ones_r = const.tile([1, 512], F16)
        nc.vector.memset(ones_r[:], 1.0)
        ones_col = const.tile([128, 1], F16)
        nc.vector.memset(ones_col[:], 1.0)
        bias_e = const.tile([128, 1], F32)
        nc.vector.memset(bias_e[:], EXP_BIAS)
        bias_0 = const.tile([128, 1], F32)
        nc.vector.memset(bias_0[:], 0.0)
        # hq duplicated on partitions 0-63 / 64-127: (128, BPC, 512)
        hq2 = const.tile([128, BPC, D], F16)
        nc.sync.dma_start(hq2[0:MQL], hq_d.rearrange("b q d -> q b d"))
        nc.sync.dma_start(hq2[MQL:128], hq_d.rearrange("b q d -> q b d"))

        st = [dict() for _ in range(BPC)]   # per-batch live tiles

        # all context loads upfront (bufs=4 -> no reuse stalls)
        for b in range(BPC):
            hc_nat = sbc.tile([128, NT, D], F16, tag="hc")
            hc_src = hc_d[b].rearrange("(t p) d -> p t d", p=128)
            for hf in range(2):
                nc.sync.dma_start(hc_nat[:, hf * 4:(hf + 1) * 4, :],
                                  hc_src[:, hf * 4:(hf + 1) * 4, :])
            st[b]["hc_nat"] = hc_nat

        def s1(b):
            """hqT/stw/sq, Hc transposes, S^T matmuls, sbank transposes."""
            v = st[b]
            hc_nat = v["hc_nat"]
            hq_r = hq2[0:MQL, b, :]
            v["hq_r"] = hq_r

            # hqT: (128, NK, MQL) via 4 PE transposes
            hqT_ps = psT.tile([128, NK, MQL], F16, tag="t")
            for k in range(NK):
                nc.tensor.transpose(
                    hqT_ps[:, k, :], hq_r[:, k * 128:(k + 1) * 128],
                    ident[0:MQL, 0:MQL])
            hqT_s = sb.tile([128, NK, MQL], F16, tag="hqT")
            nc.vector.tensor_copy(hqT_s[:], hqT_ps[:])

            # stw: cols 0:64 = hqT * Wm_k, col 64 = Wc_k
            stw = sb.tile([128, NK, MQL + 1], F16, tag="stw")
            for k in range(NK):
                nc.vector.tensor_scalar(
                    stw[:, k, 0:MQL], hqT_s[:, k, :],
                    wv[:, 8 + k, None], None, op0=ALU.mult)
            nc.vector.tensor_copy(stw[:, :, MQL], wv16[:, 0:NK])

            # sq[q] = sum_d Wq[d] Hq[q, d]  -> (1, MQL) fp32
            sq_ps = psT.tile([1, MQL], F32, tag="t")
            for k in range(NK):
                nc.tensor.matmul(sq_ps[:], wv16[:, 4 + k, None], hqT_s[:, k, :],
                                 start=(k == 0), stop=(k == NK - 1))
            sq_aug = sb.tile([1, MQL + 1], F16, tag="sq_aug")
            nc.vector.memset(sq_aug[:], 0.0)
            nc.vector.tensor_copy(sq_aug[0:1, 0:MQL], sq_ps[:])

            # hcT: (128, NK, MCL) f16 via PE transposes + 2x DVE copies
            hcT = sb.tile([128, NK, MCL], F16, tag="hcT")
            for g in range(8):
                k, hf = divmod(g, 2)
                trp = psT.tile([128, 512], F16, tag="t")
                for j in range(4):
                    t = hf * 4 + j
                    nc.tensor.transpose(
                        trp[:, j * 128:(j + 1) * 128],
                        hc_nat[:, t, k * 128:(k + 1) * 128], ident[:])
                nc.vector.tensor_copy(hcT[:, k, hf * 512:(hf + 1) * 512], trp[:])

            # S^T: (65, 1024) fp32; row 64 = sc; rank-1 adds sq per row
            sT_ps = ps.tile([MQL + 1, 2, 512], F32, tag="sT")
            for hf in range(2):
                for k in range(NK):
                    nc.tensor.matmul(
                        sT_ps[:, hf, :], stw[:, k, :],
                        hcT[:, k, hf * 512:(hf + 1) * 512],
                        start=(k == 0), stop=False)
                nc.tensor.matmul(sT_ps[:, hf, :], sq_aug[:], ones_r[:],
                                 start=False, stop=True)
            sT_s = sb.tile([MQL + 1, 2, 512], F16, tag="sT_s")
            nc.vector.tensor_copy(sT_s[:], sT_ps[:])

            # sbank: per c-tile transpose -> (128, NT, 72) f16 (cols 0:65 used)
            sbank = psA.tile([128, NT, 72], F16, tag="A")
            for t in range(NT):
                hf, j = divmod(t, 4)
                nc.tensor.transpose(
                    sbank[:, t, 0:MQL + 1],
                    sT_s[:, hf, j * 128:(j + 1) * 128],
                    ident[0:MQL + 1, 0:MQL + 1])
            v["sbank"] = sbank

        def s2(b):
            """softmax stats + q2c chain + paired wT transposes."""
            v = st[b]
            sbank, hc_nat = v["sbank"], v["hc_nat"]
            # c2q: E = exp(s - 3), dens, En
            E = sb.tile([128, NT, MQL], F16, tag="E")
            nc.scalar.activation(E[:], sbank[:, :, 0:MQL], AF.Exp,
                                 bias=bias_e[:], scale=1.0)
            mx = sb.tile([128, NT], F32, tag="mx")
            nc.vector.tensor_reduce(mx[:], sbank[:, :, 0:MQL],
                                    axis=AX.X, op=ALU.max)
            dens = sb.tile([128, NT], F32, tag="dens")
            nc.vector.tensor_reduce(dens[:], E[:], axis=AX.X, op=ALU.add)
            rec = sb.tile([128, NT], F32, tag="rec")
            nc.vector.reciprocal(rec[:], dens[:])
            En = sb.tile([128, NT, MQL], F16, tag="En")
            nc.vector.tensor_tensor(
                En[:], E[:], rec[:, :, None].broadcast_to((128, NT, MQL)),
                op=ALU.mult)
            # q2c: score = mx + sc; e2 = exp(score); U = sum_c e2[c] Hc[c,:]
            score = sb.tile([128, NT], F32, tag="score")
            nc.vector.tensor_tensor(score[:], mx[:], sbank[:, :, MQL],
                                    op=ALU.add)
            e2 = sb.tile([128, NT], F16, tag="e2")
            dsum = sb.tile([128, 1], F32, tag="dsum")
            nc.scalar.activation(e2[:], score[:], AF.Exp, bias=bias_0[:],
                                 scale=1.0, accum_out=dsum[:])
            U_ps = psT.tile([1, D], F32, tag="t")
            for t in range(NT):
                nc.tensor.matmul(U_ps[:], e2[:, t, None], hc_nat[:, t, :],
                                 start=(t == 0), stop=(t == NT - 1))
            uT = sb.tile([1, D], F16, tag="uT")
            nc.vector.tensor_copy(uT[:], U_ps[:])
            uB = sb.tile([128, D], F16, tag="uB")
            nc.gpsimd.partition_broadcast(uB[:], uT[:])
            v["uB"] = uB
            # rec2 = 1 / sum(e2) broadcast to all partitions (tanh scale)
            dsum16 = sb.tile([128, 1], F16, tag="dsum16")
            nc.vector.tensor_copy(dsum16[:], dsum[:])
            den2_ps = psT.tile([1, 1], F32, tag="t")
            nc.tensor.matmul(den2_ps[:], dsum16[:], ones_col[:],
                             start=True, stop=True)
            rec2 = sb.tile([1, 1], F32, tag="rec2")
            nc.vector.reciprocal(rec2[:], den2_ps[:])
            rec2B = sb.tile([128, 1], F32, tag="rec2B")
            nc.gpsimd.partition_broadcast(rec2B[:], rec2[:])
            v["rec2B"] = rec2B

            # wT2: paired En transposes; tile pair (2t, 2t+1) -> (128, 128)
            # rows 0:64 = q of even tile, 64:128 = q of odd tile
            wT2_ps = psA.tile([128, NT // 2, 128], F16, tag="A")
            for p in range(NT // 2):
                nc.tensor.transpose(
                    wT2_ps[:, p, :], En[:, 2 * p:2 * p + 2, :], ident[:])
            wT2 = sb.tile([128, NT // 2, 128], F16, tag="wT2")
            nc.vector.tensor_copy(wT2[:], wT2_ps[:])
            v["wT2"] = wT2

        def s3(b):
            """A matmuls (row-packed pairs) + products + tanh + store."""
            v = st[b]
            hc_nat, uB, rec2B, wT2 = (v["hc_nat"], v["uB"], v["rec2B"],
                                      v["wT2"])
            out_view = out_d[b].rearrange("(t p) j -> p t j", p=128)
            for h in range(2):
                out_t = sbo.tile([128, 4, 4 * D], F16, tag="out")
                prod = sb.tile([128, 4, 2 * D], F16, tag="prod")
                for i in range(2):
                    p = h * 2 + i          # tile pair (2p, 2p+1)
                    A_ps = psA.tile([128, 2, D], F32, tag="A")
                    nc.tensor.matmul(A_ps[:, 0, :], wT2[0:MQL, p, :],
                                     hq2[0:MQL, b, :], start=True, stop=True)
                    nc.tensor.matmul(A_ps[:, 1, :], wT2[MQL:128, p, :],
                                     hq2[MQL:128, b, :], start=True, stop=True)
                    nc.scalar.activation(out_t[:, i * 2:(i + 1) * 2, D:2 * D],
                                         A_ps[:], AF.Tanh, bias=bias_0[:],
                                         scale=1.0)
                    nc.vector.tensor_tensor(
                        prod[:, i * 2:(i + 1) * 2, 0:D], A_ps[:],
                        hc_nat[:, 2 * p:2 * p + 2, :], op=ALU.mult)
                # Hc * U on gpsimd (scale by rec2 later inside tanh)
                for i in range(2):
                    nc.gpsimd.tensor_tensor(
                        prod[:, i * 2:(i + 1) * 2, D:2 * D],
                        hc_nat[:, h * 4 + i * 2:h * 4 + (i + 1) * 2, :],
                        uB[:, None, :].broadcast_to((128, 2, D)), op=ALU.mult)
                nc.scalar.activation(out_t[:, :, 0:D],
                                     hc_nat[:, h * 4:(h + 1) * 4, :],
                                     AF.Tanh, bias=bias_0[:], scale=1.0)
                nc.scalar.activation(out_t[:, :, 2 * D:3 * D],
                                     prod[:, :, 0:D], AF.Tanh, bias=bias_0[:],
                                     scale=1.0)
                nc.scalar.activation(out_t[:, :, 3 * D:4 * D],
                                     prod[:, :, D:2 * D], AF.Tanh,
                                     bias=bias_0[:], scale=rec2B[:])
                nc.scalar.dma_start(out_view[:, h * 4:(h + 1) * 4, :],
                                    out_t[:])

        # software pipeline across batches
        s1(0)
        s1(1)
        s2(0)
        s1(2)
        s2(1)
        s3(0)
        s1(3)
        s2(2)
        s3(1)
        s2(3)
        s3(2)
        s3(3)
    nc.compile()
    return nc


_NC = None


def _get_nc():
    global _NC
    if _NC is None:
        _NC = build_nc()
    return _NC


def run(inputs: dict, trace: bool = False, tmpdir: str | None = None):
    """Shard, run on 8 cores, gather. Returns (out, BassKernelResults)."""
    from concourse.bass_utils import run_bass_kernel_spmd

    if trace:
        # the axon NTFF hook module is absent in this image; inject it
        try:
            from antenv import axon_hooks  # noqa: F401
        except ImportError:
            import types
            import antenv
            from trn_agent_boot.trn_boot import _ntff_profile_via_ctypes
            mod = types.ModuleType("antenv.axon_hooks")
            _hook = _ntff_profile_via_ctypes('/opt/axon/libaxon_pjrt.so')
            mod.get_axon_ntff_profile_hook = lambda: _hook
            mod.set_axon_ntff_profile_hook = lambda h: None
            sys.modules["antenv.axon_hooks"] = mod
            antenv.axon_hooks = mod

    Hq = np.asarray(inputs["Hq"], dtype=np.float16)
    Hc = np.asarray(inputs["Hc"], dtype=np.float16)
    W = np.ascontiguousarray(np.asarray(inputs["W"], dtype=np.float32))
    IDM = np.eye(128, dtype=np.float16)
    nc = _get_nc()
    in_maps = [
        {"hq": np.ascontiguousarray(Hq[i * BPC:(i + 1) * BPC]),
         "hc": np.ascontiguousarray(Hc[i * BPC:(i + 1) * BPC]),
         "w": W, "idm": IDM}
        for i in range(NCORES)
    ]
    br = run_bass_kernel_spmd(nc, in_maps, list(range(NCORES)), trace=trace,
                              tmpdir=tmpdir)
    out = np.concatenate([br.results[i]["out"] for i in range(NCORES)],
                         axis=0).astype(np.float32)
    return out, br


def kernel(**inputs) -> np.ndarray:
    out, _ = run(inputs, trace=False)
    return out


# revision 12
# speedup vs baseline: 1.3069x; 1.0627x over previous
"""BiDAF attention Bass kernel for Trainium2 (8 NeuronCores, batch-parallel).

Takes FULL inputs (BS=32, MCL=1024, MQL=64, d=512), shards batch across the
8 cores (4 batches/core), runs one SPMD Bass kernel, gathers the full output
(32, 1024, 2048) float32.

v3: fp16 on device (host casts in/out). S^T via big-N matmuls; softmax stats
from PE-transposed S tiles; paired En transposes + row-packed A matmuls using
a partition-duplicated Hq; products split DVE (Hc*A from PSUM) / GpSimd
(Hc*U); q2c normalization folded into the tanh scale to shorten the chain;
all loads issued upfront on the SP HWDGE ring, stores on the ACT HWDGE ring.

Self-contained: only imports concourse (available on sys.path in the
container via sitecustomize).
"""
import sys

if "/opt/trn_rl_repo" not in sys.path:
    sys.path.insert(0, "/opt/trn_rl_repo")

from contextlib import ExitStack

import numpy as np

import concourse.bass as bass
import concourse.bacc as bacc
import concourse.tile as tile
from concourse import mybir

dt = mybir.dt
AF = mybir.ActivationFunctionType
ALU = mybir.AluOpType
AX = mybir.AxisListType

NCORES = 8
BS, MCL, MQL, D = 32, 1024, 64, 512
BPC = BS // NCORES          # batches per core
NT = MCL // 128             # c-tiles per batch
NK = D // 128               # contraction chunks
F32 = dt.float32
F16 = dt.float16
EXP_BIAS = -3.0             # constant shift inside c2q softmax (exact in softmax math)


def build_nc():
    nc = bacc.Bacc("TRN2", target_bir_lowering=False)
    hq_d = nc.dram_tensor("hq", [BPC, MQL, D], F16, kind="ExternalInput")
    hc_d = nc.dram_tensor("hc", [BPC, MCL, D], F16, kind="ExternalInput")
    w_d = nc.dram_tensor("w", [3 * D, 1], F32, kind="ExternalInput")
    id_d = nc.dram_tensor("idm", [128, 128], F16, kind="ExternalInput")
    out_d = nc.dram_tensor("out", [BPC, MCL, 4 * D], F16, kind="ExternalOutput")

    with tile.TileContext(nc) as tc, ExitStack() as ctx:
        const = ctx.enter_context(tc.tile_pool(name="const", bufs=1))
        sb = ctx.enter_context(tc.tile_pool(name="sb", bufs=2))
        sbc = ctx.enter_context(tc.tile_pool(name="sbc", bufs=4))
        sbo = ctx.enter_context(tc.tile_pool(name="sbo", bufs=3))
        ps = ctx.enter_context(tc.tile_pool(name="ps", bufs=1, space="PSUM"))
        psA = ctx.enter_context(tc.tile_pool(name="psA", bufs=2, space="PSUM"))
        psT = ctx.enter_context(tc.tile_pool(name="psT", bufs=2, space="PSUM"))

        # ---- constants ----
        ident = const.tile([128, 128], F16)
        nc.sync.dma_start(ident[:], id_d[:])
        # W as (128, 12): col j holds W[j*128:(j+1)*128]; j=0..3 Wc, 4..7 Wq, 8..11 Wm
        wv = const.tile([128, 12], F32)
        nc.sync.dma_start(wv[:], w_d.rearrange("(j p) o -> p (j o)", p=128))
        wv16 = const.tile([128, 12], F16)
        nc.vector.tensor_copy(wv16[:], wv[:])
        ones_r = const.tile([1, 512], F16)
        nc.vector.memset(ones_r[:], 1.0)
        ones_col = const.tile([128, 1], F16)
        nc.vector.memset(ones_col[:], 1.0)
        bias_e = const.tile([128, 1], F32)
        nc.vector.memset(bias_e[:], EXP_BIAS)
        bias_0 = const.tile([128, 1], F32)
        nc.vector.memset(bias_0[:], 0.0)
        # hq duplicated on partitions 0-63 / 64-127: (128, BPC, 512)
        hq2 = const.tile([128, BPC, D], F16)
        nc.sync.dma_start(hq2[0:MQL], hq_d.rearrange("b q d -> q b d"))
        nc.sync.dma_start(hq2[MQL:128], hq_d.rearrange("b q d -> q b d"))

        st = [dict() for _ in range(BPC)]   # per-batch live tiles

        # all context loads upfront (bufs=4 -> no reuse stalls)
        for b in range(BPC):
            hc_nat = sbc.tile([128, NT, D], F16, tag="hc")
            hc_src = hc_d[b].rearrange("(t p) d -> p t d", p=128)
            for hf in range(2):
                nc.sync.dma_start(hc_nat[:, hf * 4:(hf + 1) * 4, :],
                                  hc_src[:, hf * 4:(hf + 1) * 4, :])
            st[b]["hc_nat"] = hc_nat

        def s1(b):
            """hqT/stw/sq, Hc transposes, S^T matmuls, sbank transposes."""
            v = st[b]
            hc_nat = v["hc_nat"]
            hq_r = hq2[0:MQL, b, :]
            v["hq_r"] = hq_r

            # hqT: (128, NK, MQL) via 4 PE transposes
            hqT_ps = psT.tile([128, NK, MQL], F16, tag="t")
            for k in range(NK):
                nc.tensor.transpose(
                    hqT_ps[:, k, :], hq_r[:, k * 128:(k + 1) * 128],
                    ident[0:MQL, 0:MQL])
            hqT_s = sb.tile([128, NK, MQL], F16, tag="hqT")
            nc.vector.tensor_copy(hqT_s[:], hqT_ps[:])

            # stw: cols 0:64 = hqT * Wm_k, col 64 = Wc_k
            stw = sb.tile([128, NK, MQL + 1], F16, tag="stw")
            for k in range(NK):
                nc.vector.tensor_scalar(
                    stw[:, k, 0:MQL], hqT_s[:, k, :],
                    wv[:, 8 + k, None], None, op0=ALU.mult)
            nc.vector.tensor_copy(stw[:, :, MQL], wv16[:, 0:NK])

            # sq[q] = sum_d Wq[d] Hq[q, d]  -> (1, MQL) fp32
            sq_ps = psT.tile([1, MQL], F32, tag="t")
            for k in range(NK):
                nc.tensor.matmul(sq_ps[:], wv16[:, 4 + k, None], hqT_s[:, k, :],
                                 start=(k == 0), stop=(k == NK - 1))
            sq_aug = sb.tile([1, MQL + 1], F16, tag="sq_aug")
            nc.vector.memset(sq_aug[:], 0.0)
            nc.vector.tensor_copy(sq_aug[0:1, 0:MQL], sq_ps[:])

            # hcT: (128, NK, MCL) f16 via PE transposes + 2x DVE copies
            hcT = sb.tile([128, NK, MCL], F16, tag="hcT")
            for g in range(8):
                k, hf = divmod(g, 2)
                trp = psT.tile([128, 512], F16, tag="t")
                for j in range(4):
                    t = hf * 4 + j
                    nc.tensor.transpose(
                        trp[:, j * 128:(j + 1) * 128],
                        hc_nat[:, t, k * 128:(k + 1) * 128], ident[:])
                nc.vector.tensor_copy(hcT[:, k, hf * 512:(hf + 1) * 512], trp[:])

            # S^T: (65, 1024) fp32; row 64 = sc; rank-1 adds sq per row
            sT_s = sb.tile([MQL + 1, 2, 512], F16, tag="sT_s")
            for hf in range(2):
                sT_ps = ps.tile([MQL + 1, 512], F32, tag="sT")
                for k in range(NK):
                    nc.tensor.matmul(
                        sT_ps[:], stw[:, k, :],
                        hcT[:, k, hf * 512:(hf + 1) * 512],
                        start=(k == 0), stop=False)
                nc.tensor.matmul(sT_ps[:], sq_aug[:], ones_r[:],
                                 start=False, stop=True)
                nc.vector.tensor_copy(sT_s[:, hf, :], sT_ps[:])

            # sbank: per c-tile transpose -> (128, NT, 72) f16 (cols 0:65 used)
            sbank = ps.tile([128, NT, 72], F16, tag="sbank")
            for t in range(NT):
                hf, j = divmod(t, 4)
                nc.tensor.transpose(
                    sbank[:, t, 0:MQL + 1],
                    sT_s[:, hf, j * 128:(j + 1) * 128],
                    ident[0:MQL + 1, 0:MQL + 1])
            v["sbank"] = sbank

        def s2(b):
            """softmax stats + q2c chain + paired wT transposes."""
            v = st[b]
            sbank, hc_nat = v["sbank"], v["hc_nat"]
            # c2q: E = exp(s - 3), dens, En
            E = sb.tile([128, NT, MQL], F16, tag="E")
            nc.scalar.activation(E[:], sbank[:, :, 0:MQL], AF.Exp,
                                 bias=bias_e[:], scale=1.0)
            mx = sb.tile([128, NT], F32, tag="mx")
            nc.vector.tensor_reduce(mx[:], sbank[:, :, 0:MQL],
                                    axis=AX.X, op=ALU.max)
            dens = sb.tile([128, NT], F32, tag="dens")
            nc.vector.tensor_reduce(dens[:], E[:], axis=AX.X, op=ALU.add)
            rec = sb.tile([128, NT], F32, tag="rec")
            nc.vector.reciprocal(rec[:], dens[:])
            En = sb.tile([128, NT, MQL], F16, tag="En")
            nc.vector.tensor_tensor(
                En[:], E[:], rec[:, :, None].broadcast_to((128, NT, MQL)),
                op=ALU.mult)
            # q2c: score = mx + sc; e2 = exp(score); U = sum_c e2[c] Hc[c,:]
            score = sb.tile([128, NT], F32, tag="score")
            nc.vector.tensor_tensor(score[:], mx[:], sbank[:, :, MQL],
                                    op=ALU.add)
            e2 = sb.tile([128, NT], F16, tag="e2")
            dsum = sb.tile([128, 1], F32, tag="dsum")
            nc.scalar.activation(e2[:], score[:], AF.Exp, bias=bias_0[:],
                                 scale=1.0, accum_out=dsum[:])
            U_ps = psT.tile([1, D], F32, tag="t")
            for t in range(NT):
                nc.tensor.matmul(U_ps[:], e2[:, t, None], hc_nat[:, t, :],
                                 start=(t == 0), stop=(t == NT - 1))
            uT = sb.tile([1, D], F16, tag="uT")
            nc.vector.tensor_copy(uT[:], U_ps[:])
            uB = sb.tile([128, D], F16, tag="uB")
            nc.gpsimd.partition_broadcast(uB[:], uT[:])
            v["uB"] = uB
            # rec2 = 1 / sum(e2) broadcast to all partitions (tanh scale)
            dsum16 = sb.tile([128, 1], F16, tag="dsum16")
            nc.vector.tensor_copy(dsum16[:], dsum[:])
            den2_ps = psT.tile([1, 1], F32, tag="t")
            nc.tensor.matmul(den2_ps[:], dsum16[:], ones_col[:],
                             start=True, stop=True)
            rec2 = sb.tile([1, 1], F32, tag="rec2")
            nc.vector.reciprocal(rec2[:], den2_ps[:])
            rec2B = sb.tile([128, 1], F32, tag="rec2B")
            nc.gpsimd.partition_broadcast(rec2B[:], rec2[:])
            v["rec2B"] = rec2B

            # wT2: paired En transposes; tile pair (2t, 2t+1) -> (128, 128)
            # rows 0:64 = q of even tile, 64:128 = q of odd tile
            wT2_ps = psA.tile([128, NT // 2, 128], F16, tag="A")
            for p in range(NT // 2):
                nc.tensor.transpose(
                    wT2_ps[:, p, :], En[:, 2 * p:2 * p + 2, :], ident[:])
            wT2 = sb.tile([128, NT // 2, 128], F16, tag="wT2")
            nc.vector.tensor_copy(wT2[:], wT2_ps[:])
            v["wT2"] = wT2

        def s3(b):
            """A matmuls (row-packed pairs) + products + tanh + store."""
            v = st[b]
            hc_nat, uB, rec2B, wT2 = (v["hc_nat"], v["uB"], v["rec2B"],
                                      v["wT2"])
            out_view = out_d[b].rearrange("(t p) j -> p t j", p=128)
            for h in range(2):
                out_t = sbo.tile([128, 4, 4 * D], F16, tag="out")
                prod = sb.tile([128, 4, 2 * D], F16, tag="prod")
                for i in range(2):
                    p = h * 2 + i          # tile pair (2p, 2p+1)
                    A_ps = psA.tile([128, 2, D], F32, tag="A")
                    nc.tensor.matmul(A_ps[:, 0, :], wT2[0:MQL, p, :],
                                     hq2[0:MQL, b, :], start=True, stop=True)
                    nc.tensor.matmul(A_ps[:, 1, :], wT2[MQL:128, p, :],
                                     hq2[MQL:128, b, :], start=True, stop=True)
                    nc.scalar.activation(out_t[:, i * 2:(i + 1) * 2, D:2 * D],
                                         A_ps[:], AF.Tanh, bias=bias_0[:],
                                         scale=1.0)
                    nc.vector.tensor_tensor(
                        prod[:, i * 2:(i + 1) * 2, 0:D], A_ps[:],
                        hc_nat[:, 2 * p:2 * p + 2, :], op=ALU.mult)
                # Hc * U on gpsimd (scale by rec2 later inside tanh)
                for i in range(2):
                    nc.gpsimd.tensor_tensor(
                        prod[:, i * 2:(i + 1) * 2, D:2 * D],
                        hc_nat[:, h * 4 + i * 2:h * 4 + (i + 1) * 2, :],
                        uB[:, None, :].broadcast_to((128, 2, D)), op=ALU.mult)
                nc.scalar.activation(out_t[:, :, 0:D],
                                     hc_nat[:, h * 4:(h + 1) * 4, :],
                                     AF.Tanh, bias=bias_0[:], scale=1.0)
                nc.scalar.activation(out_t[:, :, 2 * D:3 * D],
                                     prod[:, :, 0:D], AF.Tanh, bias=bias_0[:],
                                     scale=1.0)
                nc.scalar.activation(out_t[:, :, 3 * D:4 * D],
                                     prod[:, :, D:2 * D], AF.Tanh,
                                     bias=bias_0[:], scale=rec2B[:])
                nc.sync.dma_start(out_view[:, h * 4:(h + 1) * 4, :],
                                  out_t[:])

        # software pipeline across batches
        s1(0)
        s1(1)
        s2(0)
        s3(0)
        s1(2)
        s2(1)
        s3(1)
        s1(3)
        s2(2)
        s3(2)
        s2(3)
        s3(3)
    nc.compile()
    return nc


_NC = None


def _get_nc():
    global _NC
    if _NC is None:
        _NC = build_nc()
    return _NC


def run(inputs: dict, trace: bool = False, tmpdir: str | None = None):
    """Shard, run on 8 cores, gather. Returns (out, BassKernelResults)."""
    from concourse.bass_utils import run_bass_kernel_spmd

    if trace:
        # the axon NTFF hook module is absent in this image; inject it
        try:
            from antenv import axon_hooks  # noqa: F401
        except ImportError:
            import types
            import antenv
            from trn_agent_boot.trn_boot import _ntff_profile_via_ctypes
            mod = types.ModuleType("antenv.axon_hooks")
            _hook = _ntff_profile_via_ctypes('/opt/axon/libaxon_pjrt.so')
            mod.get_axon_ntff_profile_hook = lambda: _hook
            mod.set_axon_ntff_profile_hook = lambda h: None
            sys.modules["antenv.axon_hooks"] = mod
            antenv.axon_hooks = mod

    Hq = np.asarray(inputs["Hq"], dtype=np.float16)
    Hc = np.asarray(inputs["Hc"], dtype=np.float16)
    W = np.ascontiguousarray(np.asarray(inputs["W"], dtype=np.float32))
    IDM = np.eye(128, dtype=np.float16)
    nc = _get_nc()
    in_maps = [
        {"hq": np.ascontiguousarray(Hq[i * BPC:(i + 1) * BPC]),
         "hc": np.ascontiguousarray(Hc[i * BPC:(i + 1) * BPC]),
         "w": W, "idm": IDM}
        for i in range(NCORES)
    ]
    br = run_bass_kernel_spmd(nc, in_maps, list(range(NCORES)), trace=trace,
                              tmpdir=tmpdir)
    out = np.concatenate([br.results[i]["out"] for i in range(NCORES)],
                         axis=0).astype(np.float32)
    return out, br


def kernel(**inputs) -> np.ndarray:
    out, _ = run(inputs, trace=False)
    return out
